# revision 19
# baseline (speedup 1.0000x reference)
"""Trainium2 Bass kernel for nn_BranchingLayer (gnn_message_passing).

Computation (reference):
    parents_ftxs = x[idxs_level]                      # identity gather (arange)
    pg           = global_features[parents_idxs % B]  # random gather
    h1 = leaky_relu([parents_ftxs, pg] @ W1 + b1)
    h2 = h1 @ W2 + b2 + repeat(parents_ftxs, 2, -1)
    children[(2p+c)*B + b, f] = h2[p*B+b, c*128+f]
    out = concat([x, children])

Device strategy (8 cores, rows sharded 32768/core):
  - host: gather pg rows, pre-transpose BOTH x and pg to [feat, rows] bf16 with
    the (group, s, p) column permutation (internal row rho = s*128+p <-> DRAM
    row 4p+s within each 512-row group), so no on-device transposes are needed.
  - per 512-row group: bf16 matmuls mm1 (-> h1^T in PSUM), one-instruction
    Lrelu on ACT -> bf16 SBUF, mm2 row-major (h1^T chunks stationary, W2
    streamed; residual folded as a third accumulation against a 0/1 repeat
    matrix); one DVE copy permutes (s,c,k)->(c,s,k), casts to bf16 and stages
    the children store; stores/loads batched over group pairs (2KB/1KB descs).
  - host: upcast children bf16->f32, concat [x, children].
"""

import sys

import numpy as np

try:
    import ml_dtypes
except ImportError:
    ml_dtypes = None

if "/opt/trn_rl_repo" not in sys.path:
    sys.path.insert(0, "/opt/trn_rl_repo")

N_PARENTS = 256
BATCH = 1024
N_FEAT = 128
N_BR = 2
N_GLOBAL = 64
N_CORES = 8
ROWS = N_PARENTS * BATCH            # 262144
RPC = ROWS // N_CORES               # 32768 rows per core
CPC = RPC * N_BR                    # 65536 child rows per core
GROUP = 512                         # rows per pipeline group
N_GROUPS = RPC // GROUP             # 64
N_PAIRS = N_GROUPS // 2
HID = 256

USE_RM = True       # residual via 0/1 matrix matmul on PE (else DVE + x_rm load)
STORE_BF16 = True   # children staged/stored as bf16, upcast on host
PG_FP8 = False      # global-features path in fp8e4 + DoubleRow matmul
MM2_FP8 = True      # h1/W2 in fp8e4 + DoubleRow matmul (K=256 in one inst)
W2_COMP = True      # second DR accumulation with fp8(W2 - fp8(W2)) residual

_CACHE = {}


def _split_multiwait(nc, mybir):
    """This image's walrus accepts only one sync-wait per instruction; hoist
    extra waits onto same-engine NOPs inserted before the instruction."""
    for f in nc.m.functions:
        for bb in f.blocks:
            new_insts = []
            changed = False
            for inst in bb.instructions:
                si = inst.sync_info
                if si is not None and len(si.on_wait) > 1:
                    waits = list(si.on_wait)
                    for w in waits[:-1]:
                        new_insts.append(
                            mybir.InstNoOp(
                                name=nc.get_next_instruction_name(),
                                engine=inst.engine,
                                sync_info=mybir.SyncInfo(on_wait=[w], on_update=[]),
                            )
                        )
                    inst.sync_info = mybir.SyncInfo(
                        on_wait=[waits[-1]], on_update=list(si.on_update)
                    )
                    changed = True
                new_insts.append(inst)
            if changed:
                bb.instructions = new_insts


def _build_program(b1_zero, b2_zero, use_rm=USE_RM, store_bf16=STORE_BF16,
                   pg_fp8=PG_FP8, mm2_fp8=MM2_FP8, w2_comp=W2_COMP):
    key = ("prog", b1_zero, b2_zero, use_rm, store_bf16, pg_fp8, mm2_fp8, w2_comp)
    if key in _CACHE:
        return _CACHE[key]

    import concourse.bass as bass
    import concourse.mybir as mybir
    import concourse.tile as tile

    f32 = mybir.dt.float32
    bf16 = mybir.dt.bfloat16
    fp8 = mybir.dt.float8e4
    st_dt = bf16 if store_bf16 else f32
    pg_dt = fp8 if pg_fp8 else bf16
    h1_dt = fp8 if mm2_fp8 else bf16
    DR = mybir.MatmulPerfMode.DoubleRow

    nc = bass.Bass()
    # xt: [feat, rows] bf16, cols permuted (g, s, p)
    xt = nc.declare_dram_parameter("xt", [N_FEAT, RPC], bf16, isOutput=False)
    if pg_fp8:
        # [32, RPC*2] fp8, cols = (pair j, ktile i, n<1024); feature = i*32+q
        pgt = nc.declare_dram_parameter("pgt", [32, 2 * RPC], fp8, isOutput=False)
        w1b = nc.declare_dram_parameter("w1b", [32, 2 * HID], fp8, isOutput=False)
    else:
        pgt = nc.declare_dram_parameter("pgt", [N_GLOBAL, RPC], bf16, isOutput=False)
        w1b = nc.declare_dram_parameter("w1b", [N_GLOBAL, HID], bf16, isOutput=False)
    w1a = nc.declare_dram_parameter("w1a", [N_FEAT, HID], bf16, isOutput=False)
    if mm2_fp8:
        # [128, (1+comp)*2*HID] fp8, cols = (main/resid, ktile i, outdim);
        # W2 row = i*128 + p
        nw2 = 2 if w2_comp else 1
        w2 = nc.declare_dram_parameter("w2", [128, nw2 * 2 * HID], fp8,
                                       isOutput=False)
    else:
        w2 = nc.declare_dram_parameter("w2", [HID, HID], bf16, isOutput=False)
    b1c = nc.declare_dram_parameter("b1c", [128, 2], f32, isOutput=False)
    b2c = nc.declare_dram_parameter("b2c", [128, 2 * GROUP], f32, isOutput=False)
    ch = nc.declare_dram_parameter("ch", [CPC, N_FEAT], st_dt, isOutput=True)
    if use_rm:
        rmat = nc.declare_dram_parameter("rmat", [N_FEAT, HID], bf16, isOutput=False)
    else:
        xrm = nc.declare_dram_parameter("xrm", [128, RPC], bf16, isOutput=False)

    AF = mybir.ActivationFunctionType

    with tile.TileContext(nc) as tc:
        with (
            tc.tile_pool(name="const", bufs=1) as cpool,
            tc.tile_pool(name="xin", bufs=3) as xpool,
            tc.tile_pool(name="pg", bufs=3) as gpool,
            tc.tile_pool(name="h1", bufs=2) as h1pool,
            tc.tile_pool(name="cout", bufs=2) as opool,
            tc.tile_pool(name="ps1", bufs=2, space="PSUM") as ps1,
            tc.tile_pool(name="ps2", bufs=2, space="PSUM") as ps2,
        ):
            w1as = cpool.tile([128, HID], bf16)
            nc.sync.dma_start(w1as[:], w1a[:, :])
            if pg_fp8:
                w1bs = cpool.tile([32, 2 * HID], fp8)
                nc.sync.dma_start(w1bs[:], w1b[:, :])
            else:
                w1bs = cpool.tile([64, HID], bf16)
                nc.sync.dma_start(w1bs[:], w1b[:, :])
            if mm2_fp8:
                w2s = cpool.tile([128, (2 if w2_comp else 1) * 2 * HID], fp8)
                nc.sync.dma_start(w2s[:], w2[:, :])
            else:
                w2a = cpool.tile([128, HID], bf16)
                nc.sync.dma_start(w2a[:], w2[0:128, :])
                w2b = cpool.tile([128, HID], bf16)
                nc.sync.dma_start(w2b[:], w2[128:256, :])
            if use_rm:
                rms = cpool.tile([128, HID], bf16)
                nc.sync.dma_start(rms[:], rmat[:])
            b1s = cpool.tile([128, 2], f32)
            nc.sync.dma_start(b1s[:], b1c[:])
            b2s = cpool.tile([128, 2 * GROUP], f32)
            nc.sync.dma_start(b2s[:], b2c[:])

            def emit_loads(j):
                """Loads for pair j (groups 2j, 2j+1): [128,1024] cols."""
                xt2 = xpool.tile([128, 2 * GROUP], bf16, tag="xt2", name=f"xt2_{j}")
                nc.sync.dma_start(xt2[:, :], xt[:, j * 1024:(j + 1) * 1024])
                if pg_fp8:
                    pg2 = gpool.tile([32, 4 * GROUP], fp8, tag="pg2",
                                     name=f"pg2_{j}")
                    nc.sync.dma_start(pg2[:, :], pgt[:, j * 2048:(j + 1) * 2048])
                else:
                    pg2 = gpool.tile([64, 2 * GROUP], bf16, tag="pg2",
                                     name=f"pg2_{j}")
                    nc.sync.dma_start(pg2[:, :], pgt[:, j * 1024:(j + 1) * 1024])
                if not use_rm:
                    xr2 = xpool.tile([128, 2 * GROUP], bf16, tag="xr2",
                                     name=f"xr2_{j}")
                    nc.sync.dma_start(xr2[:, :], xrm[:, j * 1024:(j + 1) * 1024])
                else:
                    xr2 = None
                return {"xt2": xt2, "pg2": pg2, "xr2": xr2}

            def emit_mm1(g, ld):
                e = g & 1
                xs_ = ld["xt2"][:, e * GROUP:(e + 1) * GROUP]
                h1ps = ps1.tile([128, 2 * GROUP], f32, tag="h1ps", name=f"h1ps{g}")
                for m in range(2):
                    nc.tensor.matmul(
                        h1ps[:, m * GROUP:(m + 1) * GROUP],
                        w1as[:, m * 128:(m + 1) * 128], xs_,
                        start=True, stop=False,
                    )
                if pg_fp8:
                    pg_ = (ld["pg2"][:, :]
                           .rearrange("q (i n) -> q i n", i=2)
                           [:, :, e * GROUP:(e + 1) * GROUP])
                    w1b3 = w1bs[:, :].rearrange("q (i h) -> q i h", i=2)
                    for m in range(2):
                        nc.tensor.matmul(
                            h1ps[:, m * GROUP:(m + 1) * GROUP],
                            w1b3[:, :, m * 128:(m + 1) * 128], pg_,
                            start=False, stop=True, perf_mode=DR,
                        )
                else:
                    pg_ = ld["pg2"][:, e * GROUP:(e + 1) * GROUP]
                    for m in range(2):
                        nc.tensor.matmul(
                            h1ps[:, m * GROUP:(m + 1) * GROUP],
                            w1bs[:, m * 128:(m + 1) * 128], pg_,
                            start=False, stop=True,
                        )
                return h1ps

            def emit_lrelu(g, st):
                h1 = h1pool.tile([128, 2 * GROUP], h1_dt, tag="h1sb", name=f"h1sb{g}")
                if b1_zero:
                    nc.scalar.activation(
                        h1[:, :], st["h1ps"][:, :], AF.Lrelu,
                        scale=1.0, alpha=0.01,
                    )
                else:
                    for m in range(2):
                        nc.scalar.activation(
                            h1[:, m * GROUP:(m + 1) * GROUP],
                            st["h1ps"][:, m * GROUP:(m + 1) * GROUP], AF.Lrelu,
                            bias=b1s[:, m:m + 1], scale=1.0, alpha=0.01,
                        )
                st["h1"] = h1

            def emit_mm2(st):
                g, ld = st["g"], st["ld"]
                e = g & 1
                h2ps = ps2.tile([128, 2 * GROUP], f32, tag="h2ps", name=f"h2ps{g}")
                st["h2ps"] = h2ps
                h1 = st["h1"]
                if mm2_fp8:
                    h13 = h1[:, :].rearrange("p (i x) -> p i x", i=2)
                    nw2 = 2 if w2_comp else 1
                    w24 = w2s[:, :].rearrange("p (r i o) -> p r i o", r=nw2, i=2)
                    for s in range(4):
                        for r in range(nw2):
                            last = (r == nw2 - 1) and not use_rm
                            nc.tensor.matmul(
                                h2ps[:, s * HID:(s + 1) * HID],
                                h13[:, :, s * 128:(s + 1) * 128], w24[:, r],
                                start=(r == 0), stop=last, perf_mode=DR,
                            )
                        if use_rm:
                            nc.tensor.matmul(
                                h2ps[:, s * HID:(s + 1) * HID],
                                ld["xt2"][:, e * GROUP + s * 128:
                                          e * GROUP + (s + 1) * 128],
                                rms[:, :],
                                start=False, stop=True,
                            )
                else:
                    steps = [(h1, 0, w2a, True, False),
                             (h1, GROUP, w2b, False, use_rm is False)]
                    if use_rm:
                        steps.append((ld["xt2"], e * GROUP, rms, False, True))
                    for s in range(4):
                        for (src, off, wmat, st_, sp_) in steps:
                            lhs = src[:, off + s * 128: off + (s + 1) * 128]
                            nc.tensor.matmul(
                                h2ps[:, s * HID:(s + 1) * HID],
                                lhs, wmat[:, :],
                                start=st_, stop=sp_,
                            )

            def emit_tail(st, cht2):
                """DVE: permute (s,c,k)->(c,s,k), +b2, cast, stage into cht2."""
                g = st["g"]
                e = g & 1
                dst = (cht2[:, e * 2 * GROUP:(e + 1) * 2 * GROUP]
                       .rearrange("p (c t k) -> p c t k", c=2, t=4))
                src = st["h2ps"][:, :].rearrange("p (t c k) -> p c t k", t=4, c=2)
                if not use_rm:
                    xr = (st["ld"]["xr2"][:, e * GROUP:(e + 1) * GROUP]
                          .rearrange("p (t k) -> p t k", t=4))
                    for c in range(2):
                        nc.vector.tensor_add(dst[:, c], src[:, c], xr)
                    if not b2_zero:
                        b2v = b2s[:, :].rearrange("p (c t k) -> p c t k", c=2, t=4)
                        nc.vector.tensor_add(dst, dst, b2v)
                elif b2_zero:
                    nc.vector.tensor_copy(dst, src)
                else:
                    b2v = b2s[:, :].rearrange("p (c t k) -> p c t k", c=2, t=4)
                    nc.vector.tensor_add(dst, src, b2v)

            def emit_stores(j, cht2):
                """Stores for pair j: one DMA per branch c, 1024 rows each."""
                p_l = j
                v = cht2[:, :].rearrange("p (h c t k) -> p c h t k", h=2, c=2, t=4)
                for c in range(2):
                    base = (2 * p_l + c) * BATCH
                    nc.sync.dma_start(
                        ch[base:base + 1024, :]
                        .rearrange("(h p i) f -> p h i f", h=2, i=4),
                        v[:, c],
                    )

            prev = None
            cht2 = None
            lds = {}
            for g in range(N_GROUPS + 1):
                cur = None
                if g < N_GROUPS:
                    j = g // 2
                    if g & 1 == 0:
                        lds[j] = emit_loads(j)
                        cht2 = opool.tile([128, 4 * GROUP], st_dt, tag="cht2",
                                          name=f"cht2_{j}")
                        lds[j]["cht2"] = cht2
                    cur = {"g": g, "ld": lds[j], "cht2": lds[j]["cht2"]}
                    cur["h1ps"] = emit_mm1(g, lds[j])
                if prev is not None:
                    emit_mm2(prev)
                if cur is not None:
                    emit_lrelu(g, cur)
                if prev is not None:
                    emit_tail(prev, prev["cht2"])
                    if prev["g"] & 1 == 1:
                        emit_stores(prev["g"] // 2, prev["cht2"])
                prev = cur

    _split_multiwait(nc, mybir)
    _CACHE[key] = nc
    return nc


def _host_prep(x, global_features, W1, b1, W2, b2, idxs_level, parents_idxs):
    bf = ml_dtypes.bfloat16
    x = np.ascontiguousarray(np.asarray(x, dtype=np.float32))
    G = np.asarray(global_features, dtype=np.float32)
    W1 = np.asarray(W1, dtype=np.float32)
    b1 = np.asarray(b1, dtype=np.float32)
    W2 = np.asarray(W2, dtype=np.float32)
    b2 = np.asarray(b2, dtype=np.float32)
    idxs = np.asarray(idxs_level)
    pidx = np.asarray(parents_idxs)

    if np.array_equal(idxs, np.arange(ROWS, dtype=idxs.dtype)):
        xg = x
    else:  # general gather fallback (host)
        xg = np.ascontiguousarray(x[idxs])

    # device row permutation: within each 512-row group, internal row
    # rho = s*128 + p corresponds to DRAM row 4p + s; transposed inputs put
    # column g*512 + s*128 + p at that row's data.
    xt = np.ascontiguousarray(
        xg.reshape(N_CORES, N_GROUPS, 128, 4, N_FEAT)
        .transpose(0, 4, 1, 3, 2)
        .reshape(N_CORES, N_FEAT, RPC)
        .astype(bf)
    )
    pg = G[pidx % BATCH]                              # [ROWS, 64]
    pgt = np.ascontiguousarray(
        pg.reshape(N_CORES, N_GROUPS, 128, 4, N_GLOBAL).transpose(0, 4, 1, 3, 2)
        .reshape(N_CORES, N_GLOBAL, RPC)
    )
    w1a = np.ascontiguousarray(W1[0:N_FEAT, :]).astype(bf)
    w1b = W1[N_FEAT:N_FEAT + N_GLOBAL, :]
    if PG_FP8:
        f8 = ml_dtypes.float8_e4m3fn
        # feature k = i*32 + q; cols (pair j, ktile i, n<1024)
        pgt = np.ascontiguousarray(
            pgt.reshape(N_CORES, 2, 32, N_PAIRS, 1024).transpose(0, 2, 3, 1, 4)
            .reshape(N_CORES, 32, 2 * RPC)
        ).astype(f8)
        w1b = np.ascontiguousarray(
            w1b.reshape(2, 32, HID).transpose(1, 0, 2).reshape(32, 2 * HID)
        ).astype(f8)
    else:
        pgt = pgt.astype(bf)
        w1b = np.ascontiguousarray(w1b).astype(bf)
    if MM2_FP8:
        f8 = ml_dtypes.float8_e4m3fn

        def pack_w2(a):
            return (a.reshape(2, 128, HID).transpose(1, 0, 2)
                    .reshape(128, 2 * HID).astype(f8))

        w2m = pack_w2(W2)
        if W2_COMP:
            resid = W2 - pack_w2(W2).astype(np.float32).reshape(
                128, 2, HID).transpose(1, 0, 2).reshape(HID, HID)
            w2 = np.ascontiguousarray(
                np.concatenate([w2m, pack_w2(resid)], axis=1))
        else:
            w2 = np.ascontiguousarray(w2m)
    else:
        w2 = np.ascontiguousarray(W2).astype(bf)
    b1c = np.ascontiguousarray(b1.reshape(2, 128).T)  # [128, 2]
    # b2 pattern for cht cols (c, t, k): b2[c*128 + k]
    b2row = b2.reshape(2, 1, 128)[:, [0, 0, 0, 0], :].reshape(1, 2 * GROUP)
    b2c = np.ascontiguousarray(np.broadcast_to(b2row, (128, 2 * GROUP)))
    rmat = np.zeros((N_FEAT, HID), dtype=np.float32)
    k = np.arange(N_FEAT)
    rmat[k, 2 * k] = 1.0
    rmat[k, 2 * k + 1] = 1.0
    rmat = rmat.astype(bf)
    if not USE_RM:
        xrm = np.ascontiguousarray(
            xg.reshape(N_CORES, N_GROUPS, 128, 4 * N_FEAT)
            .transpose(0, 2, 1, 3)
            .reshape(N_CORES, 128, RPC)
            .astype(bf)
        )

    in_maps = []
    for c in range(N_CORES):
        m = {
            "xt": xt[c],
            "pgt": pgt[c],
            "w1a": w1a,
            "w1b": w1b,
            "w2": w2,
            "b1c": b1c,
            "b2c": b2c,
        }
        if USE_RM:
            m["rmat"] = rmat
        else:
            m["xrm"] = xrm[c]
        in_maps.append(m)
    b1_zero = not np.any(b1)
    b2_zero = not np.any(b2)
    return x, in_maps, b1_zero, b2_zero


def kernel(x, global_features, W1, b1, W2, b2, idxs_level, parents_idxs,
           _trace=False, _trace_kwargs=None):
    from concourse.bass_utils import run_bass_kernel_spmd

    x_np, in_maps, b1_zero, b2_zero = _host_prep(
        x, global_features, W1, b1, W2, b2, idxs_level, parents_idxs
    )
    nc = _build_program(b1_zero, b2_zero)
    res = run_bass_kernel_spmd(
        nc, in_maps, list(range(N_CORES)),
        trace=_trace, **(_trace_kwargs or {}),
    )
    children = np.concatenate(
        [res.results[c]["ch"] for c in range(N_CORES)], axis=0
    ).astype(np.float32)
    out = np.concatenate([x_np, children], axis=0)
    if _trace:
        kernel.last_result = res
    return out


# revision 20
# speedup vs baseline: 1.1904x; 1.1904x over previous
"""Trainium2 Bass kernel for nn_BranchingLayer (gnn_message_passing).

Computation (reference):
    parents_ftxs = x[idxs_level]                      # identity gather (arange)
    pg           = global_features[parents_idxs % B]  # random gather
    h1 = leaky_relu([parents_ftxs, pg] @ W1 + b1)
    h2 = h1 @ W2 + b2 + repeat(parents_ftxs, 2, -1)
    children[(2p+c)*B + b, f] = h2[p*B+b, c*128+f]
    out = concat([x, children])

Device strategy (8 cores, rows sharded 32768/core):
  - host: gather pg rows, pre-transpose BOTH x and pg to [feat, rows] bf16 with
    the (group, s, p) column permutation (internal row rho = s*128+p <-> DRAM
    row 4p+s within each 512-row group), so no on-device transposes are needed.
  - per 512-row group: bf16 matmuls mm1 (-> h1^T in PSUM), one-instruction
    Lrelu on ACT -> bf16 SBUF, mm2 row-major (h1^T chunks stationary, W2
    streamed; residual folded as a third accumulation against a 0/1 repeat
    matrix); one DVE copy permutes (s,c,k)->(c,s,k), casts to bf16 and stages
    the children store; stores/loads batched over group pairs (2KB/1KB descs).
  - host: upcast children bf16->f32, concat [x, children].
"""

import sys

import numpy as np

try:
    import ml_dtypes
except ImportError:
    ml_dtypes = None

if "/opt/trn_rl_repo" not in sys.path:
    sys.path.insert(0, "/opt/trn_rl_repo")

N_PARENTS = 256
BATCH = 1024
N_FEAT = 128
N_BR = 2
N_GLOBAL = 64
N_CORES = 8
ROWS = N_PARENTS * BATCH            # 262144
RPC = ROWS // N_CORES               # 32768 rows per core
CPC = RPC * N_BR                    # 65536 child rows per core
GROUP = 512                         # rows per pipeline group
N_GROUPS = RPC // GROUP             # 64
N_PAIRS = N_GROUPS // 2
HID = 256

USE_RM = True       # residual via 0/1 matrix matmul on PE (else DVE + x_rm load)
STORE_BF16 = True   # children staged/stored as bf16, upcast on host
PG_FP8 = False      # global-features path in fp8e4 + DoubleRow matmul
MM2_FP8 = True      # h1/W2 in fp8e4 + DoubleRow matmul (K=256 in one inst)
W2_COMP = True      # second DR accumulation with fp8(W2 - fp8(W2)) residual

_CACHE = {}


def _split_multiwait(nc, mybir):
    """This image's walrus accepts only one sync-wait per instruction; hoist
    extra waits onto same-engine NOPs inserted before the instruction."""
    for f in nc.m.functions:
        for bb in f.blocks:
            new_insts = []
            changed = False
            for inst in bb.instructions:
                si = inst.sync_info
                if si is not None and len(si.on_wait) > 1:
                    waits = list(si.on_wait)
                    for w in waits[:-1]:
                        new_insts.append(
                            mybir.InstNoOp(
                                name=nc.get_next_instruction_name(),
                                engine=inst.engine,
                                sync_info=mybir.SyncInfo(on_wait=[w], on_update=[]),
                            )
                        )
                    inst.sync_info = mybir.SyncInfo(
                        on_wait=[waits[-1]], on_update=list(si.on_update)
                    )
                    changed = True
                new_insts.append(inst)
            if changed:
                bb.instructions = new_insts


def _build_program(b1_zero, b2_zero, use_rm=USE_RM, store_bf16=STORE_BF16,
                   pg_fp8=PG_FP8, mm2_fp8=MM2_FP8, w2_comp=W2_COMP):
    key = ("prog", b1_zero, b2_zero, use_rm, store_bf16, pg_fp8, mm2_fp8, w2_comp)
    if key in _CACHE:
        return _CACHE[key]

    import concourse.bass as bass
    import concourse.mybir as mybir
    import concourse.tile as tile

    f32 = mybir.dt.float32
    bf16 = mybir.dt.bfloat16
    fp8 = mybir.dt.float8e4
    st_dt = bf16 if store_bf16 else f32
    pg_dt = fp8 if pg_fp8 else bf16
    h1_dt = fp8 if mm2_fp8 else bf16
    DR = mybir.MatmulPerfMode.DoubleRow

    nc = bass.Bass()
    # xt: [feat, rows] bf16, cols permuted (g, s, p)
    xt = nc.declare_dram_parameter("xt", [N_FEAT, RPC], bf16, isOutput=False)
    if pg_fp8:
        # [32, RPC*2] fp8, cols = (pair j, ktile i, n<1024); feature = i*32+q
        pgt = nc.declare_dram_parameter("pgt", [32, 2 * RPC], fp8, isOutput=False)
        w1b = nc.declare_dram_parameter("w1b", [32, 2 * HID], fp8, isOutput=False)
    else:
        pgt = nc.declare_dram_parameter("pgt", [N_GLOBAL, RPC], bf16, isOutput=False)
        w1b = nc.declare_dram_parameter("w1b", [N_GLOBAL, HID], bf16, isOutput=False)
    w1a = nc.declare_dram_parameter("w1a", [N_FEAT, HID], bf16, isOutput=False)
    if mm2_fp8:
        # [128, (1+comp)*2*HID] fp8, cols = (main/resid, ktile i, outdim);
        # W2 row = i*128 + p
        nw2 = 2 if w2_comp else 1
        w2 = nc.declare_dram_parameter("w2", [128, nw2 * 2 * HID], fp8,
                                       isOutput=False)
    else:
        w2 = nc.declare_dram_parameter("w2", [HID, HID], bf16, isOutput=False)
    b1c = nc.declare_dram_parameter("b1c", [128, 2], f32, isOutput=False)
    b2c = nc.declare_dram_parameter("b2c", [128, 2 * GROUP], f32, isOutput=False)
    ch = nc.declare_dram_parameter("ch", [CPC, N_FEAT], st_dt, isOutput=True)
    if use_rm:
        rmat = nc.declare_dram_parameter("rmat", [N_FEAT, HID], bf16, isOutput=False)
    else:
        xrm = nc.declare_dram_parameter("xrm", [128, RPC], bf16, isOutput=False)

    AF = mybir.ActivationFunctionType

    with tile.TileContext(nc) as tc:
        with (
            tc.tile_pool(name="const", bufs=1) as cpool,
            tc.tile_pool(name="xin", bufs=3) as xpool,
            tc.tile_pool(name="pg", bufs=3) as gpool,
            tc.tile_pool(name="h1", bufs=2) as h1pool,
            tc.tile_pool(name="cout", bufs=2) as opool,
            tc.tile_pool(name="ps1", bufs=2, space="PSUM") as ps1,
            tc.tile_pool(name="ps2", bufs=2, space="PSUM") as ps2,
        ):
            w1as = cpool.tile([128, HID], bf16)
            nc.sync.dma_start(w1as[:], w1a[:, :])
            if pg_fp8:
                w1bs = cpool.tile([32, 2 * HID], fp8)
                nc.sync.dma_start(w1bs[:], w1b[:, :])
            else:
                w1bs = cpool.tile([64, HID], bf16)
                nc.sync.dma_start(w1bs[:], w1b[:, :])
            if mm2_fp8:
                w2s = cpool.tile([128, (2 if w2_comp else 1) * 2 * HID], fp8)
                nc.sync.dma_start(w2s[:], w2[:, :])
            else:
                w2a = cpool.tile([128, HID], bf16)
                nc.sync.dma_start(w2a[:], w2[0:128, :])
                w2b = cpool.tile([128, HID], bf16)
                nc.sync.dma_start(w2b[:], w2[128:256, :])
            if use_rm:
                rms = cpool.tile([128, HID], bf16)
                nc.sync.dma_start(rms[:], rmat[:])
            b1s = cpool.tile([128, 2], f32)
            nc.sync.dma_start(b1s[:], b1c[:])
            b2s = cpool.tile([128, 2 * GROUP], f32)
            nc.sync.dma_start(b2s[:], b2c[:])

            def emit_loads(j):
                """Loads for pair j (groups 2j, 2j+1): [128,1024] cols."""
                xt2 = xpool.tile([128, 2 * GROUP], bf16, tag="xt2", name=f"xt2_{j}")
                nc.sync.dma_start(xt2[:, :], xt[:, j * 1024:(j + 1) * 1024])
                if pg_fp8:
                    pg2 = gpool.tile([32, 4 * GROUP], fp8, tag="pg2",
                                     name=f"pg2_{j}")
                    nc.sync.dma_start(pg2[:, :], pgt[:, j * 2048:(j + 1) * 2048])
                else:
                    pg2 = gpool.tile([64, 2 * GROUP], bf16, tag="pg2",
                                     name=f"pg2_{j}")
                    nc.sync.dma_start(pg2[:, :], pgt[:, j * 1024:(j + 1) * 1024])
                if not use_rm:
                    xr2 = xpool.tile([128, 2 * GROUP], bf16, tag="xr2",
                                     name=f"xr2_{j}")
                    nc.sync.dma_start(xr2[:, :], xrm[:, j * 1024:(j + 1) * 1024])
                else:
                    xr2 = None
                return {"xt2": xt2, "pg2": pg2, "xr2": xr2}

            def emit_mm1(g, ld):
                e = g & 1
                xs_ = ld["xt2"][:, e * GROUP:(e + 1) * GROUP]
                h1ps = ps1.tile([128, 2 * GROUP], f32, tag="h1ps", name=f"h1ps{g}")
                for m in range(2):
                    nc.tensor.matmul(
                        h1ps[:, m * GROUP:(m + 1) * GROUP],
                        w1as[:, m * 128:(m + 1) * 128], xs_,
                        start=True, stop=False,
                    )
                if pg_fp8:
                    pg_ = (ld["pg2"][:, :]
                           .rearrange("q (i n) -> q i n", i=2)
                           [:, :, e * GROUP:(e + 1) * GROUP])
                    w1b3 = w1bs[:, :].rearrange("q (i h) -> q i h", i=2)
                    for m in range(2):
                        nc.tensor.matmul(
                            h1ps[:, m * GROUP:(m + 1) * GROUP],
                            w1b3[:, :, m * 128:(m + 1) * 128], pg_,
                            start=False, stop=True, perf_mode=DR,
                        )
                else:
                    pg_ = ld["pg2"][:, e * GROUP:(e + 1) * GROUP]
                    for m in range(2):
                        nc.tensor.matmul(
                            h1ps[:, m * GROUP:(m + 1) * GROUP],
                            w1bs[:, m * 128:(m + 1) * 128], pg_,
                            start=False, stop=True,
                        )
                return h1ps

            def emit_lrelu(g, st):
                h1 = h1pool.tile([128, 2 * GROUP], h1_dt, tag="h1sb", name=f"h1sb{g}")
                if b1_zero:
                    # split along rows so mm2's first s-chunks unblock early
                    h1po = st["h1ps"][:, :].rearrange("p (m r) -> p m r", m=2)
                    h1o = h1[:, :].rearrange("p (m r) -> p m r", m=2)
                    for half in range(2):
                        rs = slice(half * 256, (half + 1) * 256)
                        nc.scalar.activation(
                            h1o[:, :, rs], h1po[:, :, rs], AF.Lrelu,
                            scale=1.0, alpha=0.01,
                        )
                else:
                    for m in range(2):
                        nc.scalar.activation(
                            h1[:, m * GROUP:(m + 1) * GROUP],
                            st["h1ps"][:, m * GROUP:(m + 1) * GROUP], AF.Lrelu,
                            bias=b1s[:, m:m + 1], scale=1.0, alpha=0.01,
                        )
                st["h1"] = h1

            def emit_mm2(st):
                g, ld = st["g"], st["ld"]
                e = g & 1
                h2ps = ps2.tile([128, 2 * GROUP], f32, tag="h2ps", name=f"h2ps{g}")
                st["h2ps"] = h2ps
                h1 = st["h1"]
                if mm2_fp8:
                    h13 = h1[:, :].rearrange("p (i x) -> p i x", i=2)
                    nw2 = 2 if w2_comp else 1
                    w24 = w2s[:, :].rearrange("p (r i o) -> p r i o", r=nw2, i=2)
                    for s in range(4):
                        for r in range(nw2):
                            last = (r == nw2 - 1) and not use_rm
                            nc.tensor.matmul(
                                h2ps[:, s * HID:(s + 1) * HID],
                                h13[:, :, s * 128:(s + 1) * 128], w24[:, r],
                                start=(r == 0), stop=last, perf_mode=DR,
                            )
                        if use_rm:
                            nc.tensor.matmul(
                                h2ps[:, s * HID:(s + 1) * HID],
                                ld["xt2"][:, e * GROUP + s * 128:
                                          e * GROUP + (s + 1) * 128],
                                rms[:, :],
                                start=False, stop=True,
                            )
                else:
                    steps = [(h1, 0, w2a, True, False),
                             (h1, GROUP, w2b, False, use_rm is False)]
                    if use_rm:
                        steps.append((ld["xt2"], e * GROUP, rms, False, True))
                    for s in range(4):
                        for (src, off, wmat, st_, sp_) in steps:
                            lhs = src[:, off + s * 128: off + (s + 1) * 128]
                            nc.tensor.matmul(
                                h2ps[:, s * HID:(s + 1) * HID],
                                lhs, wmat[:, :],
                                start=st_, stop=sp_,
                            )

            def emit_tail(st, cht2):
                """DVE: permute (s,c,k)->(c,s,k), +b2, cast, stage into cht2."""
                g = st["g"]
                e = g & 1
                dst = (cht2[:, e * 2 * GROUP:(e + 1) * 2 * GROUP]
                       .rearrange("p (c t k) -> p c t k", c=2, t=4))
                src = st["h2ps"][:, :].rearrange("p (t c k) -> p c t k", t=4, c=2)
                if not use_rm:
                    xr = (st["ld"]["xr2"][:, e * GROUP:(e + 1) * GROUP]
                          .rearrange("p (t k) -> p t k", t=4))
                    for c in range(2):
                        nc.vector.tensor_add(dst[:, c], src[:, c], xr)
                    if not b2_zero:
                        b2v = b2s[:, :].rearrange("p (c t k) -> p c t k", c=2, t=4)
                        nc.vector.tensor_add(dst, dst, b2v)
                elif b2_zero:
                    nc.vector.tensor_copy(dst, src)
                else:
                    b2v = b2s[:, :].rearrange("p (c t k) -> p c t k", c=2, t=4)
                    nc.vector.tensor_add(dst, src, b2v)

            def emit_stores(j, cht2):
                """Stores for pair j: one DMA per branch c, 1024 rows each."""
                p_l = j
                v = cht2[:, :].rearrange("p (h c t k) -> p c h t k", h=2, c=2, t=4)
                for c in range(2):
                    base = (2 * p_l + c) * BATCH
                    nc.sync.dma_start(
                        ch[base:base + 1024, :]
                        .rearrange("(h p i) f -> p h i f", h=2, i=4),
                        v[:, c],
                    )

            prev = None
            cht2 = None
            lds = {}
            for g in range(N_GROUPS + 1):
                cur = None
                if g < N_GROUPS:
                    j = g // 2
                    if g & 1 == 0:
                        lds[j] = emit_loads(j)
                        cht2 = opool.tile([128, 4 * GROUP], st_dt, tag="cht2",
                                          name=f"cht2_{j}")
                        lds[j]["cht2"] = cht2
                    cur = {"g": g, "ld": lds[j], "cht2": lds[j]["cht2"]}
                    cur["h1ps"] = emit_mm1(g, lds[j])
                if prev is not None:
                    emit_mm2(prev)
                if cur is not None:
                    emit_lrelu(g, cur)
                if prev is not None:
                    emit_tail(prev, prev["cht2"])
                    if prev["g"] & 1 == 1:
                        emit_stores(prev["g"] // 2, prev["cht2"])
                prev = cur

    _split_multiwait(nc, mybir)
    _CACHE[key] = nc
    return nc


def _host_prep(x, global_features, W1, b1, W2, b2, idxs_level, parents_idxs):
    bf = ml_dtypes.bfloat16
    x = np.ascontiguousarray(np.asarray(x, dtype=np.float32))
    G = np.asarray(global_features, dtype=np.float32)
    W1 = np.asarray(W1, dtype=np.float32)
    b1 = np.asarray(b1, dtype=np.float32)
    W2 = np.asarray(W2, dtype=np.float32)
    b2 = np.asarray(b2, dtype=np.float32)
    idxs = np.asarray(idxs_level)
    pidx = np.asarray(parents_idxs)

    if np.array_equal(idxs, np.arange(ROWS, dtype=idxs.dtype)):
        xg = x
    else:  # general gather fallback (host)
        xg = np.ascontiguousarray(x[idxs])

    # device row permutation: within each 512-row group, internal row
    # rho = s*128 + p corresponds to DRAM row 4p + s; transposed inputs put
    # column g*512 + s*128 + p at that row's data.
    xt = np.ascontiguousarray(
        xg.reshape(N_CORES, N_GROUPS, 128, 4, N_FEAT)
        .transpose(0, 4, 1, 3, 2)
        .reshape(N_CORES, N_FEAT, RPC)
        .astype(bf)
    )
    pg = G[pidx % BATCH]                              # [ROWS, 64]
    pgt = np.ascontiguousarray(
        pg.reshape(N_CORES, N_GROUPS, 128, 4, N_GLOBAL).transpose(0, 4, 1, 3, 2)
        .reshape(N_CORES, N_GLOBAL, RPC)
    )
    w1a = np.ascontiguousarray(W1[0:N_FEAT, :]).astype(bf)
    w1b = W1[N_FEAT:N_FEAT + N_GLOBAL, :]
    if PG_FP8:
        f8 = ml_dtypes.float8_e4m3fn
        # feature k = i*32 + q; cols (pair j, ktile i, n<1024)
        pgt = np.ascontiguousarray(
            pgt.reshape(N_CORES, 2, 32, N_PAIRS, 1024).transpose(0, 2, 3, 1, 4)
            .reshape(N_CORES, 32, 2 * RPC)
        ).astype(f8)
        w1b = np.ascontiguousarray(
            w1b.reshape(2, 32, HID).transpose(1, 0, 2).reshape(32, 2 * HID)
        ).astype(f8)
    else:
        pgt = pgt.astype(bf)
        w1b = np.ascontiguousarray(w1b).astype(bf)
    if MM2_FP8:
        f8 = ml_dtypes.float8_e4m3fn

        def pack_w2(a):
            return (a.reshape(2, 128, HID).transpose(1, 0, 2)
                    .reshape(128, 2 * HID).astype(f8))

        w2m = pack_w2(W2)
        if W2_COMP:
            resid = W2 - pack_w2(W2).astype(np.float32).reshape(
                128, 2, HID).transpose(1, 0, 2).reshape(HID, HID)
            w2 = np.ascontiguousarray(
                np.concatenate([w2m, pack_w2(resid)], axis=1))
        else:
            w2 = np.ascontiguousarray(w2m)
    else:
        w2 = np.ascontiguousarray(W2).astype(bf)
    b1c = np.ascontiguousarray(b1.reshape(2, 128).T)  # [128, 2]
    # b2 pattern for cht cols (c, t, k): b2[c*128 + k]
    b2row = b2.reshape(2, 1, 128)[:, [0, 0, 0, 0], :].reshape(1, 2 * GROUP)
    b2c = np.ascontiguousarray(np.broadcast_to(b2row, (128, 2 * GROUP)))
    rmat = np.zeros((N_FEAT, HID), dtype=np.float32)
    k = np.arange(N_FEAT)
    rmat[k, 2 * k] = 1.0
    rmat[k, 2 * k + 1] = 1.0
    rmat = rmat.astype(bf)
    if not USE_RM:
        xrm = np.ascontiguousarray(
            xg.reshape(N_CORES, N_GROUPS, 128, 4 * N_FEAT)
            .transpose(0, 2, 1, 3)
            .reshape(N_CORES, 128, RPC)
            .astype(bf)
        )

    in_maps = []
    for c in range(N_CORES):
        m = {
            "xt": xt[c],
            "pgt": pgt[c],
            "w1a": w1a,
            "w1b": w1b,
            "w2": w2,
            "b1c": b1c,
            "b2c": b2c,
        }
        if USE_RM:
            m["rmat"] = rmat
        else:
            m["xrm"] = xrm[c]
        in_maps.append(m)
    b1_zero = not np.any(b1)
    b2_zero = not np.any(b2)
    return x, in_maps, b1_zero, b2_zero


def kernel(x, global_features, W1, b1, W2, b2, idxs_level, parents_idxs,
           _trace=False, _trace_kwargs=None):
    from concourse.bass_utils import run_bass_kernel_spmd

    x_np, in_maps, b1_zero, b2_zero = _host_prep(
        x, global_features, W1, b1, W2, b2, idxs_level, parents_idxs
    )
    nc = _build_program(b1_zero, b2_zero)
    res = run_bass_kernel_spmd(
        nc, in_maps, list(range(N_CORES)),
        trace=_trace, **(_trace_kwargs or {}),
    )
    children = np.concatenate(
        [res.results[c]["ch"] for c in range(N_CORES)], axis=0
    ).astype(np.float32)
    out = np.concatenate([x_np, children], axis=0)
    if _trace:
        kernel.last_result = res
    return out
